# revision 2
# baseline (speedup 1.0000x reference)
"""Trainium2 Bass kernel v2 for nn_CausalContagionPredictor.

Contract: kernel(**inputs) takes FULL unsharded numpy inputs and returns
(p_final[512], arr_final[512]).

Strategy (8 cores, source-row sharded, dense 64-partition layout):
  - Node permutation: core d owns the 64 nodes own[l] = 128*(l%4) + 16*d + l//4
    (l = local id).  This makes the per-target candidate vector land in DRAM
    with a cheap [128 part, 4] -> [512] DMA (g = 4p + c), and ReduceScatter
    hands core d exactly its own nodes' candidates.
  - Layer 1 is host-decomposed as in v1: h1 = relu(S[i2-block] + bias_col),
    S bf16 resident in SBUF; bias = wp2.T @ p2 + Ab1s[:, s-block].
  - Pairs are (l, l+32).  Bank t = pairs (2t, 2t+1) = 4 sources.  Group
    u = banks (2u, 2u+1) -> one [128, 1024] fp32 PSUM tile.
  - mm2 per group: either 4 bf16 matmuls (2 per bank, tile_position packing)
    or 2 fp8e4 DoubleRow matmuls (K=256: both pairs in one pass).
  - relu2 per group: one 1024-col pass PSUM->SBUF, output fp8e4.
  - mm3: 8 fp8 DoubleRow matmuls (K=256 = 8 sources x 32 ch) accumulating a
    dense [64, 512] h3 PSUM tile (lhsT = per-group w3 block-diagonal).
  - Tail: sigmoid (ACT, bf16) -> gsc = t*pc (DVE bf16) -> 4 PE transposes ->
    free-dim max-reduce -> [128, 4] -> DMA -> ReduceScatter(max) -> p/arr
    updates ([2, 32] and [64, 1] forms).  arr bookkeeping is deferred one
    step off the critical path.  Optional PE dummy transposes keep the
    tensor engine's p-state ramped across the inter-step tail.
"""

import numpy as np
import ml_dtypes

N = 512
D = 64
STEPS = 10
N_CORES = 8
ROWS = N // N_CORES          # 64 sources per core
PAIRS = ROWS // 2            # 32 pairs (l, l+32)
GROUPS = 8                   # 8 groups x 4 pairs
BIG = 65536.0

# --- tunables ---
# relu1 engine per pair (32): D=DVE, A=ACT, G=GPSIMD/Pool
R1PAT = list("D" * 20 + "GAGG" + "GAGG" + "GGGG")
# relu2 engine per bank (16): D=DVE, A=ACT
R2PAT = list("ADDA" * 4)
# groups whose mm2 runs in fp8 DoubleRow (their relu1 tiles are written fp8)
FP8_GROUPS = {5, 6, 7}
# PE dummy transposes emitted after each step's matmuls to hold the p-state
NDUMMY = 0
# split the tail (mm3/sigmoid/gsc/transpose/reduce/ccin-dma) by column halves
TAIL_SPLIT = False
# timing-sensitivity hacks (WRONG RESULTS when nonzero) -- 1: skip middle hop,
# 2: skip entire dram roundtrip
SENS = 0
# how many groups mm3 trails behind relu2
MM3_LAG = 2
# emit all relu1 tiles before the mm2/relu2/mm3 loop (deep pools)
EAGER_RELU1 = False
# [128, 512] PSUM tiles for mm2, this many banks
PSUM_BUFS = 5

_CACHE = {}


def _srcmap(t, qh):
    """bank t, partition quarter qh -> local source id."""
    return [2 * t, 2 * t + 32, 2 * t + 1, 2 * t + 33][qh]


def _build_bass(repeat=1, single_core=False, no_cc=False):
    import concourse.bacc as bacc
    import concourse.mybir as mybir
    import concourse.tile as tile
    import concourse.bass_isa as bass_isa

    fp32 = mybir.dt.float32
    bf16 = mybir.dt.bfloat16
    fp8 = mybir.dt.float8e4
    AF = mybir.ActivationFunctionType
    OP = mybir.AluOpType
    AX = mybir.AxisListType
    PM = mybir.MatmulPerfMode

    n_cores = 1 if single_core else N_CORES
    nc = bacc.Bacc("TRN2", target_bir_lowering=False, debug=False,
                   num_devices=n_cores)

    def dram_in(name, shape, dt):
        return nc.dram_tensor(name, shape, dt, kind="ExternalInput").ap()

    S_in = dram_in("S_in", [128, PAIRS * N], bf16)
    W2blk_in = dram_in("W2blk_in", [128, 64], bf16)
    W2dr_in = dram_in("W2dr_in", [128, 256], fp8)
    LW3_in = dram_in("LW3_in", [128, GROUPS * 128], fp8)
    Ab1s_in = dram_in("Ab1s_in", [128, 32 * STEPS], fp32)
    wp2_in = dram_in("wp2_in", [2, 128], fp32)
    cg_in = dram_in("cg_in", [64, N], bf16)
    b2bc_in = dram_in("b2bc_in", [128, 1], fp32)
    b3bc_in = dram_in("b3bc_in", [64, 1], fp32)
    ident_in = dram_in("ident_in", [64, 64], bf16)
    pcol0_in = dram_in("pcol0_in", [64, 1], fp32)
    p20_in = dram_in("p20_in", [2, 32], fp32)
    arr0_in = dram_in("arr0_in", [64, 1], fp32)

    p_out = nc.dram_tensor("p_out", [ROWS], fp32, kind="ExternalOutput").ap()
    arr_out = nc.dram_tensor("arr_out", [ROWS], fp32, kind="ExternalOutput").ap()

    with tile.TileContext(nc) as tc:
        with tc.tile_pool(name="const", bufs=1) as cpool, \
             tc.tile_pool(name="bias", bufs=2) as bpool, \
             tc.tile_pool(name="h1", bufs=(34 if EAGER_RELU1 else 14)) as h1pool, \
             tc.tile_pool(name="h8", bufs=(14 if EAGER_RELU1 else 7)) as h8pool, \
             tc.tile_pool(name="r2", bufs=4) as r2pool, \
             tc.tile_pool(name="tails", bufs=3) as tpool, \
             tc.tile_pool(name="ps_mm2", bufs=PSUM_BUFS, space="PSUM") as pmm2, \
             tc.tile_pool(name="ps_h3", bufs=1, space="PSUM") as ph3, \
             tc.tile_pool(name="ps_t", bufs=1, space="PSUM") as ppt, \
             tc.tile_pool(name="dram", bufs=2, space="DRAM") as dpool:

            # ---- constants ----
            S = cpool.tile([128, PAIRS * N], bf16, name="S")
            for k in range(4):
                sl = slice(k * PAIRS * N // 4, (k + 1) * PAIRS * N // 4)
                nc.sync.dma_start(S[:, sl], S_in[:, sl])
            W2blk = cpool.tile([128, 64], bf16, name="W2blk")
            nc.sync.dma_start(W2blk[:], W2blk_in[:])
            W2dr = cpool.tile([128, 256], fp8, name="W2dr")
            nc.sync.dma_start(W2dr[:], W2dr_in[:])
            LW3 = cpool.tile([128, GROUPS * 128], fp8, name="LW3")
            nc.sync.dma_start(LW3[:], LW3_in[:])
            Ab1s = cpool.tile([128, 32 * STEPS], fp32, name="Ab1s")
            nc.sync.dma_start(Ab1s[:], Ab1s_in[:])
            wp2 = cpool.tile([2, 128], fp32, name="wp2")
            nc.sync.dma_start(wp2[:], wp2_in[:])
            cgb = cpool.tile([64, N], bf16, name="cgb")
            nc.sync.dma_start(cgb[:], cg_in[:])
            b2bc = cpool.tile([128, 1], fp32, name="b2bc")
            nc.sync.dma_start(b2bc[:], b2bc_in[:])
            b3bc = cpool.tile([64, 1], fp32, name="b3bc")
            nc.sync.dma_start(b3bc[:], b3bc_in[:])
            ident = cpool.tile([64, 64], bf16, name="ident")
            nc.sync.dma_start(ident[:], ident_in[:])

            # state ping-pong
            p_colA = cpool.tile([64, 1], fp32, name="p_colA")
            nc.sync.dma_start(p_colA[:], pcol0_in[:])
            p_colB = cpool.tile([64, 1], fp32, name="p_colB")
            p2A = cpool.tile([2, 32], fp32, name="p2A")
            nc.sync.dma_start(p2A[:], p20_in[:])
            p2B = cpool.tile([2, 32], fp32, name="p2B")
            arrA = cpool.tile([64, 1], fp32, name="arrA")
            nc.sync.dma_start(arrA[:], arr0_in[:])
            arrB = cpool.tile([64, 1], fp32, name="arrB")

            p_cur, p_nxt = p_colA, p_colB
            p2_cur, p2_nxt = p2A, p2B
            arr_cur, arr_nxt = arrA, arrB
            pending_arr = None

            for s_rep in range(STEPS * repeat):
                s = s_rep % STEPS
                # ---- per-step bias ----
                ps_b = ppt.tile([128, 32], fp32, tag="psb")
                nc.tensor.matmul(ps_b[:], wp2[:], p2_cur[:], start=True, stop=True)
                biastile = bpool.tile([128, 32], fp32, tag="biastile")
                nc.vector.tensor_tensor(
                    biastile[:, 0:2], ps_b[:, 0:2],
                    Ab1s[:, 32 * s:32 * s + 2], OP.add)

                # ---- relu1 ----
                h8tiles = {}
                bf16tiles = {}

                def relu1(i2, tiles):
                    u = i2 // 4
                    bias_ap = biastile[:, i2:i2 + 1]
                    src_ap = S[:, i2 * N:(i2 + 1) * N]
                    if u in FP8_GROUPS:
                        half = i2 % 2
                        key = (u, (i2 % 4) // 2)
                        if key not in tiles:
                            tiles[key] = h8pool.tile(
                                [128, 2 * N], fp8, tag="h8",
                                name=f"h8_{s}_{key[0]}_{key[1]}")
                        t = tiles[key]
                        dst = t[:, half * N:(half + 1) * N]
                    else:
                        t = h1pool.tile([128, N], bf16, tag="h1",
                                        name=f"h1_{s}_{i2}")
                        dst = t[:]
                    if u not in FP8_GROUPS:
                        bf16tiles[i2] = t
                    eng = R1PAT[i2]
                    if eng == "D":
                        nc.vector.tensor_scalar(
                            out=dst, in0=src_ap, scalar1=bias_ap, scalar2=0.0,
                            op0=OP.add, op1=OP.max)
                    elif eng == "G":
                        nc.gpsimd.tensor_scalar(
                            out=dst, in0=src_ap, scalar1=bias_ap, scalar2=0.0,
                            op0=OP.add, op1=OP.max)
                    else:
                        nc.scalar.activation(dst, src_ap, AF.Relu,
                                             bias=bias_ap, scale=1.0)
                    return t

                def relu1_group(u):
                    for i2 in range(4 * u, 4 * u + 4):
                        relu1(i2, h8tiles)

                ps_h3 = ph3.tile([128, N], fp32, tag="psh3")
                r2tiles = [None] * GROUPS

                def mm2_bank(u, h):
                    """bank h of group u -> its own [128, 512] PSUM tile."""
                    ps2 = pmm2.tile([128, N], fp32, tag="mm2")
                    if u in FP8_GROUPS:
                        nc.tensor.matmul(
                            ps2[:],
                            W2dr[:].rearrange("p (two m) -> p two m", two=2),
                            h8tiles[(u, h)][:].rearrange(
                                "p (two n) -> p two n", two=2),
                            start=True, stop=True, perf_mode=PM.DoubleRow)
                    else:
                        pe_, po_ = 4 * u + 2 * h, 4 * u + 2 * h + 1
                        nc.tensor.matmul(
                            ps2[0:64, :], W2blk[:],
                            bf16tiles[pe_][:], start=True, stop=True,
                            tile_position=(0, 0))
                        nc.tensor.matmul(
                            ps2[64:128, :], W2blk[:],
                            bf16tiles[po_][:], start=True, stop=True,
                            tile_position=(0, 64))
                    return ps2

                def relu2_bank(u, h, ps2):
                    if r2tiles[u] is None:
                        r2tiles[u] = r2pool.tile([128, 2 * N], fp8, tag="r2", name=f"r2_{s}_{u}")
                    r2 = r2tiles[u]
                    dst = r2[:, h * N:(h + 1) * N]
                    if R2PAT[2 * u + h] == "D":
                        nc.vector.tensor_scalar(
                            out=dst, in0=ps2[:], scalar1=b2bc[:, 0:1],
                            scalar2=0.0, op0=OP.add, op1=OP.max)
                    else:
                        nc.scalar.activation(dst, ps2[:], AF.Relu,
                                             bias=b2bc[:, 0:1], scale=1.0)

                halves = (0, 1) if TAIL_SPLIT else (None,)

                def mm3(u):
                    lw = LW3[:, 128 * u:128 * (u + 1)].rearrange(
                        "p (two m) -> p two m", two=2)
                    # group 0 must cover the full tile in one start=True pass:
                    # a later start would re-mark the whole PSUM bank row
                    # pending-zero and drop prior accumulation.
                    hs = (None,) if u == 0 else halves
                    for h in hs:
                        sl = slice(0, N) if h is None else slice(h * 256, (h + 1) * 256)
                        nc.tensor.matmul(
                            ps_h3[0:64, sl], lw,
                            r2tiles[u][:].rearrange(
                                "p (two n) -> p two n", two=2)[:, :, sl],
                            start=(u == 0), stop=(u == GROUPS - 1),
                            perf_mode=PM.DoubleRow)

                # ---- pipeline ----
                relu1(0, h8tiles)
                relu1(1, h8tiles)
                nc.vector.tensor_tensor(
                    biastile[:, 2:32], ps_b[:, 2:32],
                    Ab1s[:, 32 * s + 2:32 * (s + 1)], OP.add)
                relu1(2, h8tiles)
                relu1(3, h8tiles)
                relu1_group(1)
                if EAGER_RELU1:
                    for u in range(2, GROUPS):
                        relu1_group(u)
                # pc (bf16, depends only on p_col) -- after the first relu1
                # tiles so it does not delay mm2 start
                pc_t = bpool.tile([64, N], bf16, tag="pc")
                nc.vector.tensor_scalar(
                    out=pc_t[:], in0=cgb[:], scalar1=p_cur[:, 0:1],
                    scalar2=None, op0=OP.mult)
                for u in range(GROUPS):
                    psA = mm2_bank(u, 0)
                    if not EAGER_RELU1 and u + 2 < GROUPS:
                        relu1_group(u + 2)
                    relu2_bank(u, 0, psA)
                    psB = mm2_bank(u, 1)
                    relu2_bank(u, 1, psB)
                    if u >= MM3_LAG:
                        mm3(u - MM3_LAG)
                for u in range(GROUPS - MM3_LAG, GROUPS):
                    mm3(u)
                if pending_arr is not None:
                    pending_arr()
                    pending_arr = None

                # ---- tail (per column half when TAIL_SPLIT) ----
                t_sig = tpool.tile([64, N], bf16, tag="tsig")
                gsc = tpool.tile([64, N], bf16, tag="gsc")
                psT = ppt.tile([128, 256], bf16, tag="psT")
                cand_sb = tpool.tile([128, 4], fp32, tag="cand_sb")
                ccin = dpool.tile([N], fp32, tag="ccin")
                ccout = dpool.tile([ROWS], fp32, tag="ccout")
                for h in halves:
                    sl = slice(0, N) if h is None else slice(h * 256, (h + 1) * 256)
                    nc.scalar.activation(t_sig[:, sl], ps_h3[0:64, sl],
                                         AF.Sigmoid, bias=b3bc[:, 0:1],
                                         scale=1.0)
                    nc.vector.tensor_tensor(gsc[:, sl], t_sig[:, sl],
                                            pc_t[:, sl], OP.mult)
                    cs = (0, 1, 2, 3) if h is None else (2 * h, 2 * h + 1)
                    for c in cs:
                        nc.tensor.transpose(psT[:, 64 * c:64 * (c + 1)],
                                            gsc[:, 128 * c:128 * (c + 1)],
                                            ident[:])
                    csl = slice(cs[0], cs[-1] + 1)
                    nc.vector.tensor_reduce(
                        cand_sb[:, csl],
                        psT[:, 64 * cs[0]:64 * (cs[-1] + 1)].rearrange(
                            "p (c f) -> p c f", c=len(cs)),
                        AX.X, OP.max)
                cand2 = tpool.tile([2, 32], fp32, tag="cand2")
                cand_col = tpool.tile([64, 1], fp32, tag="cand_col")
                if SENS == 2:
                    nc.vector.tensor_copy(cand2[:, 0:4], cand_sb[0:2, 0:4])
                    nc.vector.memset(cand2[:, 4:32], 0.0)
                    nc.vector.tensor_copy(cand_col[:], cand_sb[0:64, 0:1])
                else:
                    nc.sync.dma_start(
                        ccin[:].rearrange("(p c) -> p c", c=4), cand_sb[:])
                    if SENS == 1:
                        src_cc = ccin
                    elif single_core or no_cc:
                        nc.sync.dma_start(ccout[:], ccin[0:ROWS])
                        src_cc = ccout
                    else:
                        nc.gpsimd.collective_compute(
                            "ReduceScatter", OP.max,
                            replica_groups=[list(range(N_CORES))],
                            ins=[ccin.opt()], outs=[ccout.opt()])
                        src_cc = ccout
                    nc.sync.dma_start(
                        cand2[:], src_cc[0:ROWS].rearrange("(q i) -> q i", i=32))
                    nc.sync.dma_start(cand_col[:], src_cc[0:ROWS])
                nc.vector.tensor_tensor(p2_nxt[:], p2_cur[:], cand2[:], OP.max)
                nc.vector.tensor_tensor(p_nxt[:], p_cur[:], cand_col[:], OP.max)

                def arr_update(s=s, p_old=p_cur, cc=cand_col,
                               a_cur=arr_cur, a_nxt=arr_nxt):
                    mask = tpool.tile([64, 1], fp32, tag="mask")
                    nc.vector.tensor_tensor(mask[:], cc[:], p_old[:], OP.is_gt)
                    arrtmp = tpool.tile([64, 1], fp32, tag="arrtmp")
                    nc.vector.tensor_scalar(
                        out=arrtmp[:], in0=mask[:],
                        scalar1=float(s + 1) - BIG, scalar2=BIG,
                        op0=OP.mult, op1=OP.add)
                    nc.vector.tensor_tensor(a_nxt[:], a_cur[:],
                                            arrtmp[:], OP.min)
                pending_arr = arr_update
                p_cur, p_nxt = p_nxt, p_cur
                p2_cur, p2_nxt = p2_nxt, p2_cur
                arr_cur, arr_nxt = arr_nxt, arr_cur

                # PE keep-warm dummies bridging the tail
                if NDUMMY:
                    psd = ppt.tile([64, 64], bf16, tag="psd")
                    for k in range(NDUMMY):
                        nc.tensor.transpose(psd[:], ident[:], ident[:])

            if pending_arr is not None:
                pending_arr()
                pending_arr = None

            nc.sync.dma_start(p_out[:], p_cur[:, 0:1])
            nc.sync.dma_start(arr_out[:], arr_cur[:, 0:1])

    nc.compile()
    return nc


def _host_prep(inputs):
    bf = ml_dtypes.bfloat16
    f8 = ml_dtypes.float8_e4m3
    cg = np.asarray(inputs["causal_graph"], np.float32)
    nf = np.asarray(inputs["node_features"], np.float32)
    shock = np.asarray(inputs["shock_nodes"]).astype(np.int64)
    W1 = np.asarray(inputs["W1"], np.float32)
    b1 = np.asarray(inputs["b1"], np.float32)
    W2 = np.asarray(inputs["W2"], np.float32)
    b2 = np.asarray(inputs["b2"], np.float32)
    W3 = np.asarray(inputs["W3"], np.float32)
    b3 = float(np.asarray(inputs["b3"], np.float32)[0])

    A = nf @ W1[:D]
    B = nf @ W1[D:2 * D]
    w_cg, w_p, w_s, w_f = W1[2 * D], W1[2 * D + 1], W1[2 * D + 2], W1[2 * D + 3]
    f0 = nf[:, 0]

    p0 = np.zeros(N, np.float32)
    arr0 = np.full(N, BIG, np.float32)
    p0[shock] = 1.0
    arr0[shock] = 0.0

    W2blk = np.zeros((128, 64), np.float32)
    W2blk[0:64, 0:32] = W2
    W2blk[64:128, 32:64] = W2
    W2blk = W2blk.astype(bf)

    W2dr = np.zeros((128, 2, 128), np.float32)
    for p in range(2):
        W2dr[0:64, p, 64 * p:64 * p + 32] = W2
        W2dr[64:128, p, 64 * p + 32:64 * p + 64] = W2
    W2dr = W2dr.reshape(128, 256).astype(f8)

    LW3 = np.zeros((128, GROUPS, 2, 64), np.float32)
    for u in range(GROUPS):
        for p in range(2):
            t = 2 * u + p
            for qh in range(4):
                m = _srcmap(t, qh)
                LW3[32 * qh:32 * (qh + 1), u, p, m] = W3[:, 0]
    LW3 = LW3.transpose(0, 1, 2, 3).reshape(128, GROUPS * 128).astype(f8)

    b2bc = np.tile(b2, 4).reshape(128, 1).astype(np.float32)
    ident = np.eye(64, dtype=np.float32).astype(bf)

    in_maps = []
    for d in range(N_CORES):
        own = np.array([128 * (l % 4) + 16 * d + l // 4 for l in range(ROWS)])
        cg_d = cg[own]                   # [64, 512]
        A_d = A[own]                     # [64, 64]
        f0_d = f0[own]

        S_pack = np.empty((128, PAIRS * N), np.float32)
        BT = B.T                         # [D, N]
        f0dT = np.abs(f0_d[:, None] - f0[None, :])   # [64, 512]
        for i2 in range(PAIRS):
            lo, hi = i2, i2 + 32
            blk = slice(i2 * N, (i2 + 1) * N)
            S_pack[0:64, blk] = BT + np.outer(w_cg, cg_d[lo]) + np.outer(w_f, f0dT[lo])
            S_pack[64:128, blk] = BT + np.outer(w_cg, cg_d[hi]) + np.outer(w_f, f0dT[hi])
        S_pack = S_pack.astype(bf)

        Ab1s = np.empty((128, 32 * STEPS), np.float32)
        for s in range(STEPS):
            base = b1[None, :] + (np.float32(s) / np.float32(STEPS)) * w_s[None, :]
            blk = slice(32 * s, 32 * (s + 1))
            Ab1s[0:64, blk] = (A_d[0:32] + base).T
            Ab1s[64:128, blk] = (A_d[32:64] + base).T
        wp2 = np.zeros((2, 128), np.float32)
        wp2[0, 0:64] = w_p
        wp2[1, 64:128] = w_p

        pcol0 = p0[own].reshape(64, 1).astype(np.float32)
        arr0c = arr0[own].reshape(64, 1).astype(np.float32)
        p20 = p0[own].reshape(2, 32).astype(np.float32)

        in_maps.append({
            "S_in": S_pack, "W2blk_in": W2blk, "W2dr_in": W2dr,
            "LW3_in": LW3, "Ab1s_in": Ab1s, "wp2_in": wp2,
            "cg_in": cg_d.astype(bf), "b2bc_in": b2bc,
            "b3bc_in": np.full((64, 1), b3, np.float32),
            "ident_in": ident,
            "pcol0_in": pcol0, "p20_in": p20, "arr0_in": arr0c,
        })
    return in_maps


def kernel(**inputs):
    from concourse.bass_utils import run_bass_kernel_spmd

    in_maps = _host_prep(inputs)
    if "nc" not in _CACHE:
        _CACHE["nc"] = _build_bass()
    nc = _CACHE["nc"]

    res = run_bass_kernel_spmd(nc, in_maps, core_ids=list(range(N_CORES)))
    p_full = np.empty(N, np.float32)
    arr_full = np.empty(N, np.float32)
    for d in range(N_CORES):
        own = np.array([128 * (l % 4) + 16 * d + l // 4 for l in range(ROWS)])
        p_full[own] = res.results[d]["p_out"]
        arr_full[own] = res.results[d]["arr_out"]
    arr_full = np.where(arr_full >= BIG / 2, np.inf, arr_full).astype(np.float32)
    return p_full, arr_full


# revision 4
# speedup vs baseline: 1.0565x; 1.0565x over previous
"""Trainium2 Bass kernel v2 for nn_CausalContagionPredictor.

Contract: kernel(**inputs) takes FULL unsharded numpy inputs and returns
(p_final[512], arr_final[512]).

Strategy (8 cores, source-row sharded, dense 64-partition layout):
  - Node permutation: core d owns the 64 nodes own[l] = 128*(l%4) + 16*d + l//4
    (l = local id).  This makes the per-target candidate vector land in DRAM
    with a cheap [128 part, 4] -> [512] DMA (g = 4p + c), and ReduceScatter
    hands core d exactly its own nodes' candidates.
  - Layer 1 is host-decomposed as in v1: h1 = relu(S[i2-block] + bias_col),
    S bf16 resident in SBUF; bias = wp2.T @ p2 + Ab1s[:, s-block].
  - Pairs are (l, l+32).  Bank t = pairs (2t, 2t+1) = 4 sources.  Group
    u = banks (2u, 2u+1) -> one [128, 1024] fp32 PSUM tile.
  - mm2 per group: either 4 bf16 matmuls (2 per bank, tile_position packing)
    or 2 fp8e4 DoubleRow matmuls (K=256: both pairs in one pass).
  - relu2 per group: one 1024-col pass PSUM->SBUF, output fp8e4.
  - mm3: 8 fp8 DoubleRow matmuls (K=256 = 8 sources x 32 ch) accumulating a
    dense [64, 512] h3 PSUM tile (lhsT = per-group w3 block-diagonal).
  - Tail: sigmoid (ACT, bf16) -> gsc = t*pc (DVE bf16) -> 4 PE transposes ->
    free-dim max-reduce -> [128, 4] -> DMA -> ReduceScatter(max) -> p/arr
    updates ([2, 32] and [64, 1] forms).  arr bookkeeping is deferred one
    step off the critical path.  Optional PE dummy transposes keep the
    tensor engine's p-state ramped across the inter-step tail.
"""

import numpy as np
import ml_dtypes

N = 512
D = 64
STEPS = 10
N_CORES = 8
ROWS = N // N_CORES          # 64 sources per core
PAIRS = ROWS // 2            # 32 pairs (l, l+32)
GROUPS = 8                   # 8 groups x 4 pairs
BIG = 65536.0

# --- tunables ---
# relu1 engine per pair (32): D=DVE, A=ACT, G=GPSIMD/Pool
R1PAT = list("D" * 20 + "GAGG" + "GAGG" + "GGGG")
# relu2 engine per bank (16): D=DVE, A=ACT
R2PAT = list("ADDA" * 4)
# groups whose mm2 runs in fp8 DoubleRow (their relu1 tiles are written fp8)
FP8_GROUPS = {5, 6, 7}
# PE dummy transposes emitted after each step's matmuls to hold the p-state
NDUMMY = 0
# split the tail (mm3/sigmoid/gsc/transpose/reduce/ccin-dma) by column halves
TAIL_SPLIT = False
# timing-sensitivity hacks (WRONG RESULTS when nonzero) -- 1: skip middle hop,
# 2: skip entire dram roundtrip
SENS = 0
# how many groups mm3 trails behind relu2
MM3_LAG = 3
# emit all relu1 tiles before the mm2/relu2/mm3 loop (deep pools)
EAGER_RELU1 = False
# [128, 512] PSUM tiles for mm2, this many banks
PSUM_BUFS = 5

_CACHE = {}


def _srcmap(t, qh):
    """bank t, partition quarter qh -> local source id."""
    return [2 * t, 2 * t + 32, 2 * t + 1, 2 * t + 33][qh]


def _build_bass(repeat=1, single_core=False, no_cc=False):
    import concourse.bacc as bacc
    import concourse.mybir as mybir
    import concourse.tile as tile
    import concourse.bass_isa as bass_isa

    fp32 = mybir.dt.float32
    bf16 = mybir.dt.bfloat16
    fp8 = mybir.dt.float8e4
    AF = mybir.ActivationFunctionType
    OP = mybir.AluOpType
    AX = mybir.AxisListType
    PM = mybir.MatmulPerfMode

    n_cores = 1 if single_core else N_CORES
    nc = bacc.Bacc("TRN2", target_bir_lowering=False, debug=False,
                   num_devices=n_cores)

    def dram_in(name, shape, dt):
        return nc.dram_tensor(name, shape, dt, kind="ExternalInput").ap()

    S_in = dram_in("S_in", [128, 20 * N], bf16)
    S8_in = dram_in("S8_in", [128, 12 * N], fp8)
    W2blk_in = dram_in("W2blk_in", [128, 64], bf16)
    W2dr_in = dram_in("W2dr_in", [128, 256], fp8)
    LW3_in = dram_in("LW3_in", [128, GROUPS * 128], fp8)
    Ab1s_in = dram_in("Ab1s_in", [128, 32 * STEPS], fp32)
    wp2_in = dram_in("wp2_in", [2, 128], fp32)
    cg_in = dram_in("cg_in", [64, N], bf16)
    b2bc_in = dram_in("b2bc_in", [128, 1], fp32)
    b3bc_in = dram_in("b3bc_in", [64, 1], fp32)
    ident_in = dram_in("ident_in", [64, 64], bf16)
    S0_in = dram_in("S0_in", [128, 2 * N], bf16)
    bias0_in = dram_in("bias0_in", [128, 2], fp32)
    pc0_in = dram_in("pc0_in", [4, N], bf16)
    LW30_in = dram_in("LW30_in", [128, 4], bf16)
    pcol0_in = dram_in("pcol0_in", [64, 1], fp32)
    p20_in = dram_in("p20_in", [2, 32], fp32)
    arr0_in = dram_in("arr0_in", [64, 1], fp32)

    p_out = nc.dram_tensor("p_out", [ROWS], fp32, kind="ExternalOutput").ap()
    arr_out = nc.dram_tensor("arr_out", [ROWS], fp32, kind="ExternalOutput").ap()

    with tile.TileContext(nc) as tc:
        with tc.tile_pool(name="const", bufs=1) as cpool, \
             tc.tile_pool(name="bias", bufs=2) as bpool, \
             tc.tile_pool(name="h1", bufs=(34 if EAGER_RELU1 else 14)) as h1pool, \
             tc.tile_pool(name="h8", bufs=(14 if EAGER_RELU1 else 7)) as h8pool, \
             tc.tile_pool(name="r2", bufs=4) as r2pool, \
             tc.tile_pool(name="tails", bufs=3) as tpool, \
             tc.tile_pool(name="ps_mm2", bufs=PSUM_BUFS, space="PSUM") as pmm2, \
             tc.tile_pool(name="ps_h3", bufs=1, space="PSUM") as ph3, \
             tc.tile_pool(name="ps_t", bufs=1, space="PSUM") as ppt, \
             tc.tile_pool(name="dram", bufs=2, space="DRAM") as dpool:

            # ---- constants (ordered so step 0 can start early) ----
            S = cpool.tile([128, 20 * N], bf16, name="S")
            S8 = cpool.tile([128, 12 * N], fp8, name="S8")
            nc.sync.dma_start(S[:, 0:8 * N], S_in[:, 0:8 * N])
            p2A = cpool.tile([2, 32], fp32, name="p2A")
            nc.sync.dma_start(p2A[:], p20_in[:])
            wp2 = cpool.tile([2, 128], fp32, name="wp2")
            nc.sync.dma_start(wp2[:], wp2_in[:])
            Ab1s = cpool.tile([128, 32 * STEPS], fp32, name="Ab1s")
            nc.sync.dma_start(Ab1s[:], Ab1s_in[:])
            W2blk = cpool.tile([128, 64], bf16, name="W2blk")
            nc.sync.dma_start(W2blk[:], W2blk_in[:])
            p_colA = cpool.tile([64, 1], fp32, name="p_colA")
            nc.sync.dma_start(p_colA[:], pcol0_in[:])
            cgb = cpool.tile([64, N], bf16, name="cgb")
            nc.sync.dma_start(cgb[:], cg_in[:])
            b2bc = cpool.tile([128, 1], fp32, name="b2bc")
            nc.sync.dma_start(b2bc[:], b2bc_in[:])
            nc.sync.dma_start(S[:, 8 * N:14 * N], S_in[:, 8 * N:14 * N])
            W2dr = cpool.tile([128, 256], fp8, name="W2dr")
            nc.sync.dma_start(W2dr[:], W2dr_in[:])
            LW3 = cpool.tile([128, GROUPS * 128], fp8, name="LW3")
            nc.sync.dma_start(LW3[:], LW3_in[:])
            nc.sync.dma_start(S[:, 14 * N:20 * N], S_in[:, 14 * N:20 * N])
            nc.sync.dma_start(S8[:, 0:6 * N], S8_in[:, 0:6 * N])
            b3bc = cpool.tile([64, 1], fp32, name="b3bc")
            nc.sync.dma_start(b3bc[:], b3bc_in[:])
            ident = cpool.tile([64, 64], bf16, name="ident")
            nc.sync.dma_start(ident[:], ident_in[:])
            arrA = cpool.tile([64, 1], fp32, name="arrA")
            nc.sync.dma_start(arrA[:], arr0_in[:])
            nc.sync.dma_start(S8[:, 6 * N:12 * N], S8_in[:, 6 * N:12 * N])
            S0 = cpool.tile([128, 2 * N], bf16, name="S0")
            nc.sync.dma_start(S0[:], S0_in[:])
            bias0 = cpool.tile([128, 2], fp32, name="bias0")
            nc.sync.dma_start(bias0[:], bias0_in[:])
            pc0 = cpool.tile([4, N], bf16, name="pc0")
            nc.sync.dma_start(pc0[:], pc0_in[:])
            LW30 = cpool.tile([128, 4], bf16, name="LW30")
            nc.sync.dma_start(LW30[:], LW30_in[:])

            p_colB = cpool.tile([64, 1], fp32, name="p_colB")
            p2B = cpool.tile([2, 32], fp32, name="p2B")
            arrB = cpool.tile([64, 1], fp32, name="arrB")

            p_cur, p_nxt = p_colA, p_colB
            p2_cur, p2_nxt = p2A, p2B
            arr_cur, arr_nxt = arrA, arrB
            pending_arr = None

            # ---- step 0: only the <=4 shock sources have p>0; all cores
            # compute the full 4-source edge set redundantly (identical
            # inputs), the ReduceScatter of identical vectors just slices.
            h1a = h1pool.tile([128, N], bf16, tag="h1", name="h1s0a")
            nc.vector.tensor_scalar(
                out=h1a[:], in0=S0[:, 0:N], scalar1=bias0[:, 0:1],
                scalar2=0.0, op0=OP.add, op1=OP.max)
            h1b = h1pool.tile([128, N], bf16, tag="h1", name="h1s0b")
            nc.vector.tensor_scalar(
                out=h1b[:], in0=S0[:, N:2 * N], scalar1=bias0[:, 1:2],
                scalar2=0.0, op0=OP.add, op1=OP.max)
            ps20 = pmm2.tile([128, N], fp32, tag="mm2")
            nc.tensor.matmul(ps20[0:64, :], W2blk[:], h1a[:],
                             start=True, stop=True, tile_position=(0, 0))
            nc.tensor.matmul(ps20[64:128, :], W2blk[:], h1b[:],
                             start=True, stop=True, tile_position=(0, 64))
            r20 = h1pool.tile([128, N], bf16, tag="h1", name="r2s0")
            nc.vector.tensor_scalar(
                out=r20[:], in0=ps20[:], scalar1=b2bc[:, 0:1],
                scalar2=0.0, op0=OP.add, op1=OP.max)
            ps_h30 = ph3.tile([128, N], fp32, tag="psh3")
            nc.tensor.matmul(ps_h30[0:4, :], LW30[:], r20[:],
                             start=True, stop=True)
            t_sig0 = tpool.tile([64, N], bf16, tag="tsig")
            nc.scalar.activation(t_sig0[0:4, :], ps_h30[0:4, :], AF.Sigmoid,
                                 bias=b3bc[0:4, 0:1], scale=1.0)
            gsc0 = tpool.tile([64, N], bf16, tag="gsc")
            nc.vector.tensor_tensor(gsc0[0:4, :], t_sig0[0:4, :], pc0[:],
                                    OP.mult)
            psT0 = ppt.tile([128, 256], bf16, tag="psT")
            for c in range(4):
                nc.tensor.transpose(psT0[:, 4 * c:4 * c + 4],
                                    gsc0[0:4, 128 * c:128 * (c + 1)],
                                    ident[0:4, 0:4])
            cand_sb0 = tpool.tile([128, 4], fp32, tag="cand_sb")
            nc.vector.tensor_reduce(
                cand_sb0[:],
                psT0[:, 0:16].rearrange("p (c f) -> p c f", c=4),
                AX.X, OP.max)
            ccin0 = dpool.tile([N], fp32, tag="ccin")
            ccout0 = dpool.tile([ROWS], fp32, tag="ccout")
            nc.sync.dma_start(
                ccin0[:].rearrange("(p c) -> p c", c=4), cand_sb0[:])
            if single_core or no_cc:
                nc.sync.dma_start(ccout0[:], ccin0[0:ROWS])
            else:
                nc.gpsimd.collective_compute(
                    "ReduceScatter", OP.max,
                    replica_groups=[list(range(N_CORES))],
                    ins=[ccin0.opt()], outs=[ccout0.opt()])
            cand20 = tpool.tile([2, 32], fp32, tag="cand2")
            nc.sync.dma_start(
                cand20[:], ccout0[:].rearrange("(q i) -> q i", i=32))
            cand_col0 = tpool.tile([64, 1], fp32, tag="cand_col")
            nc.sync.dma_start(cand_col0[:], ccout0[:])
            nc.vector.tensor_tensor(p2_nxt[:], p2_cur[:], cand20[:], OP.max)
            nc.vector.tensor_tensor(p_nxt[:], p_cur[:], cand_col0[:], OP.max)

            def arr_update0(p_old=p_cur, cc=cand_col0, a_cur=arr_cur,
                            a_nxt=arr_nxt):
                mask = tpool.tile([64, 1], fp32, tag="mask")
                nc.vector.tensor_tensor(mask[:], cc[:], p_old[:], OP.is_gt)
                arrtmp = tpool.tile([64, 1], fp32, tag="arrtmp")
                nc.vector.tensor_scalar(
                    out=arrtmp[:], in0=mask[:],
                    scalar1=1.0 - BIG, scalar2=BIG,
                    op0=OP.mult, op1=OP.add)
                nc.vector.tensor_tensor(a_nxt[:], a_cur[:],
                                        arrtmp[:], OP.min)
            pending_arr = arr_update0
            p_cur, p_nxt = p_nxt, p_cur
            p2_cur, p2_nxt = p2_nxt, p2_cur
            arr_cur, arr_nxt = arr_nxt, arr_cur

            for s_rep in range(1, STEPS * repeat):
                s = s_rep % STEPS
                # ---- per-step bias ----
                ps_b = ppt.tile([128, 32], fp32, tag="psb")
                nc.tensor.matmul(ps_b[:], wp2[:], p2_cur[:], start=True, stop=True)
                biastile = bpool.tile([128, 32], fp32, tag="biastile")
                nc.vector.tensor_tensor(
                    biastile[:, 0:2], ps_b[:, 0:2],
                    Ab1s[:, 32 * s:32 * s + 2], OP.add)

                # ---- relu1 ----
                h8tiles = {}
                bf16tiles = {}

                def relu1(i2, tiles):
                    u = i2 // 4
                    bias_ap = biastile[:, i2:i2 + 1]
                    if u in FP8_GROUPS:
                        k8 = i2 - 4 * min(FP8_GROUPS)
                        src_ap = S8[:, k8 * N:(k8 + 1) * N]
                    else:
                        src_ap = S[:, i2 * N:(i2 + 1) * N]
                    if u in FP8_GROUPS:
                        half = i2 % 2
                        key = (u, (i2 % 4) // 2)
                        if key not in tiles:
                            tiles[key] = h8pool.tile(
                                [128, 2 * N], fp8, tag="h8",
                                name=f"h8_{s}_{key[0]}_{key[1]}")
                        t = tiles[key]
                        dst = t[:, half * N:(half + 1) * N]
                    else:
                        t = h1pool.tile([128, N], bf16, tag="h1",
                                        name=f"h1_{s}_{i2}")
                        dst = t[:]
                    if u not in FP8_GROUPS:
                        bf16tiles[i2] = t
                    eng = R1PAT[i2]
                    if eng == "D":
                        nc.vector.tensor_scalar(
                            out=dst, in0=src_ap, scalar1=bias_ap, scalar2=0.0,
                            op0=OP.add, op1=OP.max)
                    elif eng == "G":
                        nc.gpsimd.tensor_scalar(
                            out=dst, in0=src_ap, scalar1=bias_ap, scalar2=0.0,
                            op0=OP.add, op1=OP.max)
                    else:
                        nc.scalar.activation(dst, src_ap, AF.Relu,
                                             bias=bias_ap, scale=1.0)
                    return t

                def relu1_group(u):
                    for i2 in range(4 * u, 4 * u + 4):
                        relu1(i2, h8tiles)

                ps_h3 = ph3.tile([128, N], fp32, tag="psh3")
                r2tiles = [None] * GROUPS

                def mm2_bank(u, h):
                    """bank h of group u -> its own [128, 512] PSUM tile."""
                    ps2 = pmm2.tile([128, N], fp32, tag="mm2")
                    if u in FP8_GROUPS:
                        nc.tensor.matmul(
                            ps2[:],
                            W2dr[:].rearrange("p (two m) -> p two m", two=2),
                            h8tiles[(u, h)][:].rearrange(
                                "p (two n) -> p two n", two=2),
                            start=True, stop=True, perf_mode=PM.DoubleRow)
                    else:
                        pe_, po_ = 4 * u + 2 * h, 4 * u + 2 * h + 1
                        nc.tensor.matmul(
                            ps2[0:64, :], W2blk[:],
                            bf16tiles[pe_][:], start=True, stop=True,
                            tile_position=(0, 0))
                        nc.tensor.matmul(
                            ps2[64:128, :], W2blk[:],
                            bf16tiles[po_][:], start=True, stop=True,
                            tile_position=(0, 64))
                    return ps2

                def relu2_bank(u, h, ps2):
                    if r2tiles[u] is None:
                        r2tiles[u] = r2pool.tile([128, 2 * N], fp8, tag="r2", name=f"r2_{s}_{u}")
                    r2 = r2tiles[u]
                    dst = r2[:, h * N:(h + 1) * N]
                    if R2PAT[2 * u + h] == "D":
                        nc.vector.tensor_scalar(
                            out=dst, in0=ps2[:], scalar1=b2bc[:, 0:1],
                            scalar2=0.0, op0=OP.add, op1=OP.max)
                    else:
                        nc.scalar.activation(dst, ps2[:], AF.Relu,
                                             bias=b2bc[:, 0:1], scale=1.0)

                halves = (0, 1) if TAIL_SPLIT else (None,)

                def mm3(u):
                    lw = LW3[:, 128 * u:128 * (u + 1)].rearrange(
                        "p (two m) -> p two m", two=2)
                    # group 0 must cover the full tile in one start=True pass:
                    # a later start would re-mark the whole PSUM bank row
                    # pending-zero and drop prior accumulation.
                    hs = (None,) if u == 0 else halves
                    for h in hs:
                        sl = slice(0, N) if h is None else slice(h * 256, (h + 1) * 256)
                        nc.tensor.matmul(
                            ps_h3[0:64, sl], lw,
                            r2tiles[u][:].rearrange(
                                "p (two n) -> p two n", two=2)[:, :, sl],
                            start=(u == 0), stop=(u == GROUPS - 1),
                            perf_mode=PM.DoubleRow)

                # ---- pipeline ----
                relu1(0, h8tiles)
                relu1(1, h8tiles)
                nc.vector.tensor_tensor(
                    biastile[:, 2:32], ps_b[:, 2:32],
                    Ab1s[:, 32 * s + 2:32 * (s + 1)], OP.add)
                relu1(2, h8tiles)
                relu1(3, h8tiles)
                relu1_group(1)
                if EAGER_RELU1:
                    for u in range(2, GROUPS):
                        relu1_group(u)
                # pc (bf16, depends only on p_col) -- after the first relu1
                # tiles so it does not delay mm2 start
                pc_t = bpool.tile([64, N], bf16, tag="pc")
                nc.vector.tensor_scalar(
                    out=pc_t[:], in0=cgb[:], scalar1=p_cur[:, 0:1],
                    scalar2=None, op0=OP.mult)
                for u in range(GROUPS):
                    psA = mm2_bank(u, 0)
                    if not EAGER_RELU1 and u + 2 < GROUPS:
                        relu1_group(u + 2)
                    relu2_bank(u, 0, psA)
                    psB = mm2_bank(u, 1)
                    relu2_bank(u, 1, psB)
                    if u >= MM3_LAG:
                        mm3(u - MM3_LAG)
                for u in range(GROUPS - MM3_LAG, GROUPS):
                    mm3(u)
                if pending_arr is not None:
                    pending_arr()
                    pending_arr = None

                # ---- tail (per column half when TAIL_SPLIT) ----
                t_sig = tpool.tile([64, N], bf16, tag="tsig")
                gsc = tpool.tile([64, N], bf16, tag="gsc")
                psT = ppt.tile([128, 256], bf16, tag="psT")
                cand_sb = tpool.tile([128, 4], fp32, tag="cand_sb")
                ccin = dpool.tile([N], fp32, tag="ccin")
                ccout = dpool.tile([ROWS], fp32, tag="ccout")
                for h in halves:
                    sl = slice(0, N) if h is None else slice(h * 256, (h + 1) * 256)
                    nc.scalar.activation(t_sig[:, sl], ps_h3[0:64, sl],
                                         AF.Sigmoid, bias=b3bc[:, 0:1],
                                         scale=1.0)
                    nc.vector.tensor_tensor(gsc[:, sl], t_sig[:, sl],
                                            pc_t[:, sl], OP.mult)
                    cs = (0, 1, 2, 3) if h is None else (2 * h, 2 * h + 1)
                    for c in cs:
                        nc.tensor.transpose(psT[:, 64 * c:64 * (c + 1)],
                                            gsc[:, 128 * c:128 * (c + 1)],
                                            ident[:])
                    csl = slice(cs[0], cs[-1] + 1)
                    nc.vector.tensor_reduce(
                        cand_sb[:, csl],
                        psT[:, 64 * cs[0]:64 * (cs[-1] + 1)].rearrange(
                            "p (c f) -> p c f", c=len(cs)),
                        AX.X, OP.max)
                cand2 = tpool.tile([2, 32], fp32, tag="cand2")
                cand_col = tpool.tile([64, 1], fp32, tag="cand_col")
                if SENS == 2:
                    nc.vector.tensor_copy(cand2[:, 0:4], cand_sb[0:2, 0:4])
                    nc.vector.memset(cand2[:, 4:32], 0.0)
                    nc.vector.tensor_copy(cand_col[:], cand_sb[0:64, 0:1])
                else:
                    nc.sync.dma_start(
                        ccin[:].rearrange("(p c) -> p c", c=4), cand_sb[:])
                    if SENS == 1:
                        src_cc = ccin
                    elif single_core or no_cc:
                        nc.sync.dma_start(ccout[:], ccin[0:ROWS])
                        src_cc = ccout
                    else:
                        nc.gpsimd.collective_compute(
                            "ReduceScatter", OP.max,
                            replica_groups=[list(range(N_CORES))],
                            ins=[ccin.opt()], outs=[ccout.opt()])
                        src_cc = ccout
                    nc.sync.dma_start(
                        cand2[:], src_cc[0:ROWS].rearrange("(q i) -> q i", i=32))
                    nc.sync.dma_start(cand_col[:], src_cc[0:ROWS])
                nc.vector.tensor_tensor(p2_nxt[:], p2_cur[:], cand2[:], OP.max)
                nc.vector.tensor_tensor(p_nxt[:], p_cur[:], cand_col[:], OP.max)

                def arr_update(s=s, p_old=p_cur, cc=cand_col,
                               a_cur=arr_cur, a_nxt=arr_nxt):
                    mask = tpool.tile([64, 1], fp32, tag="mask")
                    nc.vector.tensor_tensor(mask[:], cc[:], p_old[:], OP.is_gt)
                    arrtmp = tpool.tile([64, 1], fp32, tag="arrtmp")
                    nc.vector.tensor_scalar(
                        out=arrtmp[:], in0=mask[:],
                        scalar1=float(s + 1) - BIG, scalar2=BIG,
                        op0=OP.mult, op1=OP.add)
                    nc.vector.tensor_tensor(a_nxt[:], a_cur[:],
                                            arrtmp[:], OP.min)
                pending_arr = arr_update
                p_cur, p_nxt = p_nxt, p_cur
                p2_cur, p2_nxt = p2_nxt, p2_cur
                arr_cur, arr_nxt = arr_nxt, arr_cur

                # PE keep-warm dummies bridging the tail
                if NDUMMY:
                    psd = ppt.tile([64, 64], bf16, tag="psd")
                    for k in range(NDUMMY):
                        nc.tensor.transpose(psd[:], ident[:], ident[:])

            if pending_arr is not None:
                pending_arr()
                pending_arr = None

            nc.sync.dma_start(p_out[:], p_cur[:, 0:1])
            nc.sync.dma_start(arr_out[:], arr_cur[:, 0:1])

    nc.compile()
    return nc


def _host_prep(inputs):
    bf = ml_dtypes.bfloat16
    f8 = ml_dtypes.float8_e4m3
    cg = np.asarray(inputs["causal_graph"], np.float32)
    nf = np.asarray(inputs["node_features"], np.float32)
    shock = np.asarray(inputs["shock_nodes"]).astype(np.int64)
    W1 = np.asarray(inputs["W1"], np.float32)
    b1 = np.asarray(inputs["b1"], np.float32)
    W2 = np.asarray(inputs["W2"], np.float32)
    b2 = np.asarray(inputs["b2"], np.float32)
    W3 = np.asarray(inputs["W3"], np.float32)
    b3 = float(np.asarray(inputs["b3"], np.float32)[0])

    A = nf @ W1[:D]
    B = nf @ W1[D:2 * D]
    w_cg, w_p, w_s, w_f = W1[2 * D], W1[2 * D + 1], W1[2 * D + 2], W1[2 * D + 3]
    f0 = nf[:, 0]

    p0 = np.zeros(N, np.float32)
    arr0 = np.full(N, BIG, np.float32)
    p0[shock] = 1.0
    arr0[shock] = 0.0

    W2blk = np.zeros((128, 64), np.float32)
    W2blk[0:64, 0:32] = W2
    W2blk[64:128, 32:64] = W2
    W2blk = W2blk.astype(bf)

    W2dr = np.zeros((128, 2, 128), np.float32)
    for p in range(2):
        W2dr[0:64, p, 64 * p:64 * p + 32] = W2
        W2dr[64:128, p, 64 * p + 32:64 * p + 64] = W2
    W2dr = W2dr.reshape(128, 256).astype(f8)

    LW3 = np.zeros((128, GROUPS, 2, 64), np.float32)
    for u in range(GROUPS):
        for p in range(2):
            t = 2 * u + p
            for qh in range(4):
                m = _srcmap(t, qh)
                LW3[32 * qh:32 * (qh + 1), u, p, m] = W3[:, 0]
    LW3 = LW3.transpose(0, 1, 2, 3).reshape(128, GROUPS * 128).astype(f8)

    b2bc = np.tile(b2, 4).reshape(128, 1).astype(np.float32)
    ident = np.eye(64, dtype=np.float32).astype(bf)

    # step-0 shock-source tiles (identical on every core)
    sh = shock.astype(np.int64)
    f0d_sh = np.abs(f0[sh][:, None] - f0[None, :])          # [4, 512]
    S0 = np.empty((128, 2 * N), np.float32)
    for pair in range(2):
        for half in range(2):
            r = sh[2 * pair + half]
            rows = slice(64 * half, 64 * half + 64)
            S0[rows, pair * N:(pair + 1) * N] = (
                B.T + np.outer(w_cg, cg[r]) + np.outer(w_f, f0d_sh[2 * pair + half]))
    S0 = S0.astype(bf)
    c_sh = A[sh] + b1[None, :] + 1.0 * w_p[None, :]          # [4, 64]
    bias0 = np.empty((128, 2), np.float32)
    for pair in range(2):
        bias0[0:64, pair] = c_sh[2 * pair]
        bias0[64:128, pair] = c_sh[2 * pair + 1]
    pc0 = cg[sh].astype(bf)                                  # [4, 512]
    LW30 = np.zeros((128, 4), np.float32)
    for q in range(4):
        LW30[32 * q:32 * (q + 1), q] = W3[:, 0]
    LW30 = LW30.astype(bf)

    in_maps = []
    for d in range(N_CORES):
        own = np.array([128 * (l % 4) + 16 * d + l // 4 for l in range(ROWS)])
        cg_d = cg[own]                   # [64, 512]
        A_d = A[own]                     # [64, 64]
        f0_d = f0[own]

        S_pack = np.empty((128, PAIRS * N), np.float32)
        BT = B.T                         # [D, N]
        f0dT = np.abs(f0_d[:, None] - f0[None, :])   # [64, 512]
        for i2 in range(PAIRS):
            lo, hi = i2, i2 + 32
            blk = slice(i2 * N, (i2 + 1) * N)
            S_pack[0:64, blk] = BT + np.outer(w_cg, cg_d[lo]) + np.outer(w_f, f0dT[lo])
            S_pack[64:128, blk] = BT + np.outer(w_cg, cg_d[hi]) + np.outer(w_f, f0dT[hi])
        S_bf = S_pack[:, 0:20 * N].astype(bf)
        S_f8 = S_pack[:, 20 * N:32 * N].astype(f8)

        Ab1s = np.empty((128, 32 * STEPS), np.float32)
        for s in range(STEPS):
            base = b1[None, :] + (np.float32(s) / np.float32(STEPS)) * w_s[None, :]
            blk = slice(32 * s, 32 * (s + 1))
            Ab1s[0:64, blk] = (A_d[0:32] + base).T
            Ab1s[64:128, blk] = (A_d[32:64] + base).T
        wp2 = np.zeros((2, 128), np.float32)
        wp2[0, 0:64] = w_p
        wp2[1, 64:128] = w_p

        pcol0 = p0[own].reshape(64, 1).astype(np.float32)
        arr0c = arr0[own].reshape(64, 1).astype(np.float32)
        p20 = p0[own].reshape(2, 32).astype(np.float32)

        in_maps.append({
            "S_in": S_bf, "S8_in": S_f8, "W2blk_in": W2blk, "W2dr_in": W2dr,
            "S0_in": S0, "bias0_in": bias0, "pc0_in": pc0, "LW30_in": LW30,
            "LW3_in": LW3, "Ab1s_in": Ab1s, "wp2_in": wp2,
            "cg_in": cg_d.astype(bf), "b2bc_in": b2bc,
            "b3bc_in": np.full((64, 1), b3, np.float32),
            "ident_in": ident,
            "pcol0_in": pcol0, "p20_in": p20, "arr0_in": arr0c,
        })
    return in_maps


def kernel(**inputs):
    from concourse.bass_utils import run_bass_kernel_spmd

    in_maps = _host_prep(inputs)
    if "nc" not in _CACHE:
        _CACHE["nc"] = _build_bass()
    nc = _CACHE["nc"]

    res = run_bass_kernel_spmd(nc, in_maps, core_ids=list(range(N_CORES)))
    p_full = np.empty(N, np.float32)
    arr_full = np.empty(N, np.float32)
    for d in range(N_CORES):
        own = np.array([128 * (l % 4) + 16 * d + l // 4 for l in range(ROWS)])
        p_full[own] = res.results[d]["p_out"]
        arr_full[own] = res.results[d]["arr_out"]
    arr_full = np.where(arr_full >= BIG / 2, np.inf, arr_full).astype(np.float32)
    return p_full, arr_full


# revision 5
# speedup vs baseline: 1.0640x; 1.0071x over previous
"""Trainium2 Bass kernel v2 for nn_CausalContagionPredictor.

Contract: kernel(**inputs) takes FULL unsharded numpy inputs and returns
(p_final[512], arr_final[512]).

Strategy (8 cores, source-row sharded, dense 64-partition layout):
  - Node permutation: core d owns the 64 nodes own[l] = 128*(l%4) + 16*d + l//4
    (l = local id).  This makes the per-target candidate vector land in DRAM
    with a cheap [128 part, 4] -> [512] DMA (g = 4p + c), and ReduceScatter
    hands core d exactly its own nodes' candidates.
  - Layer 1 is host-decomposed as in v1: h1 = relu(S[i2-block] + bias_col),
    S bf16 resident in SBUF; bias = wp2.T @ p2 + Ab1s[:, s-block].
  - Pairs are (l, l+32).  Bank t = pairs (2t, 2t+1) = 4 sources.  Group
    u = banks (2u, 2u+1) -> one [128, 1024] fp32 PSUM tile.
  - mm2 per group: either 4 bf16 matmuls (2 per bank, tile_position packing)
    or 2 fp8e4 DoubleRow matmuls (K=256: both pairs in one pass).
  - relu2 per group: one 1024-col pass PSUM->SBUF, output fp8e4.
  - mm3: 8 fp8 DoubleRow matmuls (K=256 = 8 sources x 32 ch) accumulating a
    dense [64, 512] h3 PSUM tile (lhsT = per-group w3 block-diagonal).
  - Tail: sigmoid (ACT, bf16) -> gsc = t*pc (DVE bf16) -> 4 PE transposes ->
    free-dim max-reduce -> [128, 4] -> DMA -> ReduceScatter(max) -> p/arr
    updates ([2, 32] and [64, 1] forms).  arr bookkeeping is deferred one
    step off the critical path.  Optional PE dummy transposes keep the
    tensor engine's p-state ramped across the inter-step tail.
"""

import numpy as np
import ml_dtypes

N = 512
D = 64
STEPS = 10
N_CORES = 8
ROWS = N // N_CORES          # 64 sources per core
PAIRS = ROWS // 2            # 32 pairs (l, l+32)
GROUPS = 8                   # 8 groups x 4 pairs
BIG = 65536.0

# --- tunables ---
# relu1 engine per pair (32): D=DVE, A=ACT, G=GPSIMD/Pool
R1PAT = list("D" * 20 + "GAGG" + "GAGG" + "GAGG")
# relu2 engine per bank (16): D=DVE, A=ACT
R2PAT = list("DADAADDAADDAADAD")
# groups whose mm2 runs in fp8 DoubleRow (their relu1 tiles are written fp8)
FP8_GROUPS = {5, 6, 7}
# PE dummy transposes emitted after each step's matmuls to hold the p-state
NDUMMY = 0
# split the tail (mm3/sigmoid/gsc/transpose/reduce/ccin-dma) by column halves
TAIL_SPLIT = False
# timing-sensitivity hacks (WRONG RESULTS when nonzero) -- 1: skip middle hop,
# 2: skip entire dram roundtrip
SENS = 0
# how many groups mm3 trails behind relu2
MM3_LAG = 3
# emit all relu1 tiles before the mm2/relu2/mm3 loop (deep pools)
EAGER_RELU1 = False
# [128, 512] PSUM tiles for mm2, this many banks
PSUM_BUFS = 5

_CACHE = {}


def _srcmap(t, qh):
    """bank t, partition quarter qh -> local source id."""
    return [2 * t, 2 * t + 32, 2 * t + 1, 2 * t + 33][qh]


def _build_bass(repeat=1, single_core=False, no_cc=False):
    import concourse.bacc as bacc
    import concourse.mybir as mybir
    import concourse.tile as tile
    import concourse.bass_isa as bass_isa

    fp32 = mybir.dt.float32
    bf16 = mybir.dt.bfloat16
    fp8 = mybir.dt.float8e4
    AF = mybir.ActivationFunctionType
    OP = mybir.AluOpType
    AX = mybir.AxisListType
    PM = mybir.MatmulPerfMode

    n_cores = 1 if single_core else N_CORES
    nc = bacc.Bacc("TRN2", target_bir_lowering=False, debug=False,
                   num_devices=n_cores)

    def dram_in(name, shape, dt):
        return nc.dram_tensor(name, shape, dt, kind="ExternalInput").ap()

    S_in = dram_in("S_in", [128, 20 * N], bf16)
    S8_in = dram_in("S8_in", [128, 12 * N], fp8)
    W2blk_in = dram_in("W2blk_in", [128, 64], bf16)
    W2dr_in = dram_in("W2dr_in", [128, 256], fp8)
    LW3_in = dram_in("LW3_in", [128, GROUPS * 128], fp8)
    Ab1s_in = dram_in("Ab1s_in", [128, 32 * STEPS], fp32)
    wp2_in = dram_in("wp2_in", [2, 128], fp32)
    cg_in = dram_in("cg_in", [64, N], bf16)
    b2bc_in = dram_in("b2bc_in", [128, 1], fp32)
    b3bc_in = dram_in("b3bc_in", [64, 1], fp32)
    ident_in = dram_in("ident_in", [64, 64], bf16)
    S0_in = dram_in("S0_in", [128, 2 * N], bf16)
    bias0_in = dram_in("bias0_in", [128, 2], fp32)
    pc0_in = dram_in("pc0_in", [4, N], bf16)
    LW30_in = dram_in("LW30_in", [128, 4], bf16)
    pcol0_in = dram_in("pcol0_in", [64, 1], fp32)
    p20_in = dram_in("p20_in", [2, 32], fp32)
    arr0_in = dram_in("arr0_in", [64, 1], fp32)

    p_out = nc.dram_tensor("p_out", [ROWS], fp32, kind="ExternalOutput").ap()
    arr_out = nc.dram_tensor("arr_out", [ROWS], fp32, kind="ExternalOutput").ap()

    with tile.TileContext(nc) as tc:
        with tc.tile_pool(name="const", bufs=1) as cpool, \
             tc.tile_pool(name="bias", bufs=2) as bpool, \
             tc.tile_pool(name="h1", bufs=(34 if EAGER_RELU1 else 14)) as h1pool, \
             tc.tile_pool(name="h8", bufs=(14 if EAGER_RELU1 else 7)) as h8pool, \
             tc.tile_pool(name="r2", bufs=4) as r2pool, \
             tc.tile_pool(name="tails", bufs=3) as tpool, \
             tc.tile_pool(name="ps_mm2", bufs=PSUM_BUFS, space="PSUM") as pmm2, \
             tc.tile_pool(name="ps_h3", bufs=1, space="PSUM") as ph3, \
             tc.tile_pool(name="ps_t", bufs=1, space="PSUM") as ppt, \
             tc.tile_pool(name="dram", bufs=2, space="DRAM") as dpool:

            # ---- constants needed by step 0 only ----
            S = cpool.tile([128, 20 * N], bf16, name="S")
            S8 = cpool.tile([128, 12 * N], fp8, name="S8")
            S0 = cpool.tile([128, 2 * N], bf16, name="S0")
            nc.sync.dma_start(S0[:], S0_in[:])
            bias0 = cpool.tile([128, 2], fp32, name="bias0")
            nc.sync.dma_start(bias0[:], bias0_in[:])
            pc0 = cpool.tile([4, N], bf16, name="pc0")
            nc.sync.dma_start(pc0[:], pc0_in[:])
            LW30 = cpool.tile([128, 4], bf16, name="LW30")
            nc.sync.dma_start(LW30[:], LW30_in[:])
            W2blk = cpool.tile([128, 64], bf16, name="W2blk")
            nc.sync.dma_start(W2blk[:], W2blk_in[:])
            b2bc = cpool.tile([128, 1], fp32, name="b2bc")
            nc.sync.dma_start(b2bc[:], b2bc_in[:])
            b3bc = cpool.tile([64, 1], fp32, name="b3bc")
            nc.sync.dma_start(b3bc[:], b3bc_in[:])
            ident = cpool.tile([64, 64], bf16, name="ident")
            nc.sync.dma_start(ident[:], ident_in[:])
            p2A = cpool.tile([2, 32], fp32, name="p2A")
            nc.sync.dma_start(p2A[:], p20_in[:])
            p_colA = cpool.tile([64, 1], fp32, name="p_colA")
            nc.sync.dma_start(p_colA[:], pcol0_in[:])
            wp2 = cpool.tile([2, 128], fp32, name="wp2")
            Ab1s = cpool.tile([128, 32 * STEPS], fp32, name="Ab1s")
            cgb = cpool.tile([64, N], bf16, name="cgb")
            W2dr = cpool.tile([128, 256], fp8, name="W2dr")
            LW3 = cpool.tile([128, GROUPS * 128], fp8, name="LW3")
            arrA = cpool.tile([64, 1], fp32, name="arrA")

            def load_rest_a():
                """emitted between the step-0 collective and its cand loads:
                the HWDGE slots fit inside the collective's latency window."""
                nc.sync.dma_start(wp2[:], wp2_in[:])
                nc.sync.dma_start(Ab1s[:], Ab1s_in[:])

            def load_rest_b():
                """emitted after the step-0 cand loads: streams during the
                step-1 body."""
                nc.sync.dma_start(arrA[:], arr0_in[:])
                nc.sync.dma_start(cgb[:], cg_in[:])
                nc.sync.dma_start(W2dr[:], W2dr_in[:])
                nc.sync.dma_start(LW3[:], LW3_in[:])
                nc.sync.dma_start(S[:, 0:8 * N], S_in[:, 0:8 * N])
                nc.sync.dma_start(S8[:, 0:6 * N], S8_in[:, 0:6 * N])
                nc.sync.dma_start(S8[:, 6 * N:12 * N], S8_in[:, 6 * N:12 * N])
                nc.sync.dma_start(S[:, 8 * N:14 * N], S_in[:, 8 * N:14 * N])
                nc.sync.dma_start(S[:, 14 * N:20 * N], S_in[:, 14 * N:20 * N])

            p_colB = cpool.tile([64, 1], fp32, name="p_colB")
            p2B = cpool.tile([2, 32], fp32, name="p2B")
            arrB = cpool.tile([64, 1], fp32, name="arrB")

            p_cur, p_nxt = p_colA, p_colB
            p2_cur, p2_nxt = p2A, p2B
            arr_cur, arr_nxt = arrA, arrB
            pending_arr = None

            # ---- step 0: only the <=4 shock sources have p>0; all cores
            # compute the full 4-source edge set redundantly (identical
            # inputs), the ReduceScatter of identical vectors just slices.
            h1a = h1pool.tile([128, N], bf16, tag="h1", name="h1s0a")
            nc.vector.tensor_scalar(
                out=h1a[:], in0=S0[:, 0:N], scalar1=bias0[:, 0:1],
                scalar2=0.0, op0=OP.add, op1=OP.max)
            h1b = h1pool.tile([128, N], bf16, tag="h1", name="h1s0b")
            nc.vector.tensor_scalar(
                out=h1b[:], in0=S0[:, N:2 * N], scalar1=bias0[:, 1:2],
                scalar2=0.0, op0=OP.add, op1=OP.max)
            ps20 = pmm2.tile([128, N], fp32, tag="mm2")
            nc.tensor.matmul(ps20[0:64, :], W2blk[:], h1a[:],
                             start=True, stop=True, tile_position=(0, 0))
            nc.tensor.matmul(ps20[64:128, :], W2blk[:], h1b[:],
                             start=True, stop=True, tile_position=(0, 64))
            r20 = h1pool.tile([128, N], bf16, tag="h1", name="r2s0")
            nc.vector.tensor_scalar(
                out=r20[:], in0=ps20[:], scalar1=b2bc[:, 0:1],
                scalar2=0.0, op0=OP.add, op1=OP.max)
            ps_h30 = ph3.tile([128, N], fp32, tag="psh3")
            nc.tensor.matmul(ps_h30[0:4, :], LW30[:], r20[:],
                             start=True, stop=True)
            t_sig0 = tpool.tile([64, N], bf16, tag="tsig")
            nc.scalar.activation(t_sig0[0:4, :], ps_h30[0:4, :], AF.Sigmoid,
                                 bias=b3bc[0:4, 0:1], scale=1.0)
            gsc0 = tpool.tile([64, N], bf16, tag="gsc")
            nc.vector.tensor_tensor(gsc0[0:4, :], t_sig0[0:4, :], pc0[:],
                                    OP.mult)
            psT0 = ppt.tile([128, 256], bf16, tag="psT")
            for c in range(4):
                nc.tensor.transpose(psT0[:, 4 * c:4 * c + 4],
                                    gsc0[0:4, 128 * c:128 * (c + 1)],
                                    ident[0:4, 0:4])
            cand_sb0 = tpool.tile([128, 4], fp32, tag="cand_sb")
            nc.vector.tensor_reduce(
                cand_sb0[:],
                psT0[:, 0:16].rearrange("p (c f) -> p c f", c=4),
                AX.X, OP.max)
            ccin0 = dpool.tile([N], fp32, tag="ccin")
            ccout0 = dpool.tile([ROWS], fp32, tag="ccout")
            nc.sync.dma_start(
                ccin0[:].rearrange("(p c) -> p c", c=4), cand_sb0[:])
            if single_core or no_cc:
                nc.sync.dma_start(ccout0[:], ccin0[0:ROWS])
            else:
                nc.gpsimd.collective_compute(
                    "ReduceScatter", OP.max,
                    replica_groups=[list(range(N_CORES))],
                    ins=[ccin0.opt()], outs=[ccout0.opt()])
            load_rest_a()
            cand20 = tpool.tile([2, 32], fp32, tag="cand2")
            nc.sync.dma_start(
                cand20[:], ccout0[:].rearrange("(q i) -> q i", i=32))
            cand_col0 = tpool.tile([64, 1], fp32, tag="cand_col")
            nc.sync.dma_start(cand_col0[:], ccout0[:])
            load_rest_b()
            nc.vector.tensor_tensor(p2_nxt[:], p2_cur[:], cand20[:], OP.max)
            nc.vector.tensor_tensor(p_nxt[:], p_cur[:], cand_col0[:], OP.max)

            def arr_update0(p_old=p_cur, cc=cand_col0, a_cur=arr_cur,
                            a_nxt=arr_nxt):
                mask = tpool.tile([64, 1], fp32, tag="mask")
                nc.vector.tensor_tensor(mask[:], cc[:], p_old[:], OP.is_gt)
                arrtmp = tpool.tile([64, 1], fp32, tag="arrtmp")
                nc.vector.tensor_scalar(
                    out=arrtmp[:], in0=mask[:],
                    scalar1=1.0 - BIG, scalar2=BIG,
                    op0=OP.mult, op1=OP.add)
                nc.vector.tensor_tensor(a_nxt[:], a_cur[:],
                                        arrtmp[:], OP.min)
            pending_arr = arr_update0
            p_cur, p_nxt = p_nxt, p_cur
            p2_cur, p2_nxt = p2_nxt, p2_cur
            arr_cur, arr_nxt = arr_nxt, arr_cur

            for s_rep in range(1, STEPS * repeat):
                s = s_rep % STEPS
                # ---- per-step bias ----
                ps_b = ppt.tile([128, 32], fp32, tag="psb")
                nc.tensor.matmul(ps_b[:], wp2[:], p2_cur[:], start=True, stop=True)
                biastile = bpool.tile([128, 32], fp32, tag="biastile")
                nc.vector.tensor_tensor(
                    biastile[:, 0:2], ps_b[:, 0:2],
                    Ab1s[:, 32 * s:32 * s + 2], OP.add)

                # ---- relu1 ----
                h8tiles = {}
                bf16tiles = {}

                def relu1(i2, tiles):
                    u = i2 // 4
                    bias_ap = biastile[:, i2:i2 + 1]
                    if u in FP8_GROUPS:
                        k8 = i2 - 4 * min(FP8_GROUPS)
                        src_ap = S8[:, k8 * N:(k8 + 1) * N]
                    else:
                        src_ap = S[:, i2 * N:(i2 + 1) * N]
                    if u in FP8_GROUPS:
                        half = i2 % 2
                        key = (u, (i2 % 4) // 2)
                        if key not in tiles:
                            tiles[key] = h8pool.tile(
                                [128, 2 * N], fp8, tag="h8",
                                name=f"h8_{s}_{key[0]}_{key[1]}")
                        t = tiles[key]
                        dst = t[:, half * N:(half + 1) * N]
                    else:
                        t = h1pool.tile([128, N], bf16, tag="h1",
                                        name=f"h1_{s}_{i2}")
                        dst = t[:]
                    if u not in FP8_GROUPS:
                        bf16tiles[i2] = t
                    eng = R1PAT[i2]
                    if eng == "D":
                        nc.vector.tensor_scalar(
                            out=dst, in0=src_ap, scalar1=bias_ap, scalar2=0.0,
                            op0=OP.add, op1=OP.max)
                    elif eng == "G":
                        nc.gpsimd.tensor_scalar(
                            out=dst, in0=src_ap, scalar1=bias_ap, scalar2=0.0,
                            op0=OP.add, op1=OP.max)
                    else:
                        nc.scalar.activation(dst, src_ap, AF.Relu,
                                             bias=bias_ap, scale=1.0)
                    return t

                def relu1_group(u):
                    for i2 in range(4 * u, 4 * u + 4):
                        relu1(i2, h8tiles)

                ps_h3 = ph3.tile([128, N], fp32, tag="psh3")
                r2tiles = [None] * GROUPS

                def mm2_bank(u, h):
                    """bank h of group u -> its own [128, 512] PSUM tile."""
                    ps2 = pmm2.tile([128, N], fp32, tag="mm2")
                    if u in FP8_GROUPS:
                        nc.tensor.matmul(
                            ps2[:],
                            W2dr[:].rearrange("p (two m) -> p two m", two=2),
                            h8tiles[(u, h)][:].rearrange(
                                "p (two n) -> p two n", two=2),
                            start=True, stop=True, perf_mode=PM.DoubleRow)
                    else:
                        pe_, po_ = 4 * u + 2 * h, 4 * u + 2 * h + 1
                        nc.tensor.matmul(
                            ps2[0:64, :], W2blk[:],
                            bf16tiles[pe_][:], start=True, stop=True,
                            tile_position=(0, 0))
                        nc.tensor.matmul(
                            ps2[64:128, :], W2blk[:],
                            bf16tiles[po_][:], start=True, stop=True,
                            tile_position=(0, 64))
                    return ps2

                def relu2_bank(u, h, ps2):
                    if r2tiles[u] is None:
                        r2tiles[u] = r2pool.tile([128, 2 * N], fp8, tag="r2", name=f"r2_{s}_{u}")
                    r2 = r2tiles[u]
                    dst = r2[:, h * N:(h + 1) * N]
                    if R2PAT[2 * u + h] == "D":
                        nc.vector.tensor_scalar(
                            out=dst, in0=ps2[:], scalar1=b2bc[:, 0:1],
                            scalar2=0.0, op0=OP.add, op1=OP.max)
                    else:
                        nc.scalar.activation(dst, ps2[:], AF.Relu,
                                             bias=b2bc[:, 0:1], scale=1.0)

                halves = (0, 1) if TAIL_SPLIT else (None,)

                def mm3(u):
                    lw = LW3[:, 128 * u:128 * (u + 1)].rearrange(
                        "p (two m) -> p two m", two=2)
                    # group 0 must cover the full tile in one start=True pass:
                    # a later start would re-mark the whole PSUM bank row
                    # pending-zero and drop prior accumulation.
                    hs = (None,) if u == 0 else halves
                    for h in hs:
                        sl = slice(0, N) if h is None else slice(h * 256, (h + 1) * 256)
                        nc.tensor.matmul(
                            ps_h3[0:64, sl], lw,
                            r2tiles[u][:].rearrange(
                                "p (two n) -> p two n", two=2)[:, :, sl],
                            start=(u == 0), stop=(u == GROUPS - 1),
                            perf_mode=PM.DoubleRow)

                # ---- pipeline ----
                relu1(0, h8tiles)
                relu1(1, h8tiles)
                nc.vector.tensor_tensor(
                    biastile[:, 2:32], ps_b[:, 2:32],
                    Ab1s[:, 32 * s + 2:32 * (s + 1)], OP.add)
                relu1(2, h8tiles)
                relu1(3, h8tiles)
                relu1_group(1)
                if EAGER_RELU1:
                    for u in range(2, GROUPS):
                        relu1_group(u)
                # pc (bf16, depends only on p_col) -- after the first relu1
                # tiles so it does not delay mm2 start
                pc_t = bpool.tile([64, N], bf16, tag="pc")
                nc.vector.tensor_scalar(
                    out=pc_t[:], in0=cgb[:], scalar1=p_cur[:, 0:1],
                    scalar2=None, op0=OP.mult)
                for u in range(GROUPS):
                    psA = mm2_bank(u, 0)
                    if not EAGER_RELU1 and u + 2 < GROUPS:
                        relu1_group(u + 2)
                    relu2_bank(u, 0, psA)
                    psB = mm2_bank(u, 1)
                    relu2_bank(u, 1, psB)
                    if u >= MM3_LAG:
                        mm3(u - MM3_LAG)
                for u in range(GROUPS - MM3_LAG, GROUPS):
                    mm3(u)
                if pending_arr is not None:
                    pending_arr()
                    pending_arr = None

                # ---- tail (per column half when TAIL_SPLIT) ----
                t_sig = tpool.tile([64, N], bf16, tag="tsig")
                gsc = tpool.tile([64, N], bf16, tag="gsc")
                psT = ppt.tile([128, 256], bf16, tag="psT")
                cand_sb = tpool.tile([128, 4], fp32, tag="cand_sb")
                ccin = dpool.tile([N], fp32, tag="ccin")
                ccout = dpool.tile([ROWS], fp32, tag="ccout")
                for h in halves:
                    sl = slice(0, N) if h is None else slice(h * 256, (h + 1) * 256)
                    nc.scalar.activation(t_sig[:, sl], ps_h3[0:64, sl],
                                         AF.Sigmoid, bias=b3bc[:, 0:1],
                                         scale=1.0)
                    nc.vector.tensor_tensor(gsc[:, sl], t_sig[:, sl],
                                            pc_t[:, sl], OP.mult)
                    cs = (0, 1, 2, 3) if h is None else (2 * h, 2 * h + 1)
                    for c in cs:
                        nc.tensor.transpose(psT[:, 64 * c:64 * (c + 1)],
                                            gsc[:, 128 * c:128 * (c + 1)],
                                            ident[:])
                    csl = slice(cs[0], cs[-1] + 1)
                    nc.vector.tensor_reduce(
                        cand_sb[:, csl],
                        psT[:, 64 * cs[0]:64 * (cs[-1] + 1)].rearrange(
                            "p (c f) -> p c f", c=len(cs)),
                        AX.X, OP.max)
                cand2 = tpool.tile([2, 32], fp32, tag="cand2")
                cand_col = tpool.tile([64, 1], fp32, tag="cand_col")
                if SENS == 2:
                    nc.vector.tensor_copy(cand2[:, 0:4], cand_sb[0:2, 0:4])
                    nc.vector.memset(cand2[:, 4:32], 0.0)
                    nc.vector.tensor_copy(cand_col[:], cand_sb[0:64, 0:1])
                else:
                    nc.sync.dma_start(
                        ccin[:].rearrange("(p c) -> p c", c=4), cand_sb[:])
                    if SENS == 1:
                        src_cc = ccin
                    elif single_core or no_cc:
                        nc.sync.dma_start(ccout[:], ccin[0:ROWS])
                        src_cc = ccout
                    else:
                        nc.gpsimd.collective_compute(
                            "ReduceScatter", OP.max,
                            replica_groups=[list(range(N_CORES))],
                            ins=[ccin.opt()], outs=[ccout.opt()])
                        src_cc = ccout
                    nc.sync.dma_start(
                        cand2[:], src_cc[0:ROWS].rearrange("(q i) -> q i", i=32))
                    nc.sync.dma_start(cand_col[:], src_cc[0:ROWS])
                nc.vector.tensor_tensor(p2_nxt[:], p2_cur[:], cand2[:], OP.max)
                nc.vector.tensor_tensor(p_nxt[:], p_cur[:], cand_col[:], OP.max)

                def arr_update(s=s, p_old=p_cur, cc=cand_col,
                               a_cur=arr_cur, a_nxt=arr_nxt):
                    mask = tpool.tile([64, 1], fp32, tag="mask")
                    nc.vector.tensor_tensor(mask[:], cc[:], p_old[:], OP.is_gt)
                    arrtmp = tpool.tile([64, 1], fp32, tag="arrtmp")
                    nc.vector.tensor_scalar(
                        out=arrtmp[:], in0=mask[:],
                        scalar1=float(s + 1) - BIG, scalar2=BIG,
                        op0=OP.mult, op1=OP.add)
                    nc.vector.tensor_tensor(a_nxt[:], a_cur[:],
                                            arrtmp[:], OP.min)
                pending_arr = arr_update
                p_cur, p_nxt = p_nxt, p_cur
                p2_cur, p2_nxt = p2_nxt, p2_cur
                arr_cur, arr_nxt = arr_nxt, arr_cur

                # PE keep-warm dummies bridging the tail
                if NDUMMY:
                    psd = ppt.tile([64, 64], bf16, tag="psd")
                    for k in range(NDUMMY):
                        nc.tensor.transpose(psd[:], ident[:], ident[:])

            if pending_arr is not None:
                pending_arr()
                pending_arr = None

            nc.sync.dma_start(p_out[:], p_cur[:, 0:1])
            nc.sync.dma_start(arr_out[:], arr_cur[:, 0:1])

    nc.compile()
    return nc


def _host_prep(inputs):
    bf = ml_dtypes.bfloat16
    f8 = ml_dtypes.float8_e4m3
    cg = np.asarray(inputs["causal_graph"], np.float32)
    nf = np.asarray(inputs["node_features"], np.float32)
    shock = np.asarray(inputs["shock_nodes"]).astype(np.int64)
    W1 = np.asarray(inputs["W1"], np.float32)
    b1 = np.asarray(inputs["b1"], np.float32)
    W2 = np.asarray(inputs["W2"], np.float32)
    b2 = np.asarray(inputs["b2"], np.float32)
    W3 = np.asarray(inputs["W3"], np.float32)
    b3 = float(np.asarray(inputs["b3"], np.float32)[0])

    A = nf @ W1[:D]
    B = nf @ W1[D:2 * D]
    w_cg, w_p, w_s, w_f = W1[2 * D], W1[2 * D + 1], W1[2 * D + 2], W1[2 * D + 3]
    f0 = nf[:, 0]

    p0 = np.zeros(N, np.float32)
    arr0 = np.full(N, BIG, np.float32)
    p0[shock] = 1.0
    arr0[shock] = 0.0

    W2blk = np.zeros((128, 64), np.float32)
    W2blk[0:64, 0:32] = W2
    W2blk[64:128, 32:64] = W2
    W2blk = W2blk.astype(bf)

    W2dr = np.zeros((128, 2, 128), np.float32)
    for p in range(2):
        W2dr[0:64, p, 64 * p:64 * p + 32] = W2
        W2dr[64:128, p, 64 * p + 32:64 * p + 64] = W2
    W2dr = W2dr.reshape(128, 256).astype(f8)

    LW3 = np.zeros((128, GROUPS, 2, 64), np.float32)
    for u in range(GROUPS):
        for p in range(2):
            t = 2 * u + p
            for qh in range(4):
                m = _srcmap(t, qh)
                LW3[32 * qh:32 * (qh + 1), u, p, m] = W3[:, 0]
    LW3 = LW3.transpose(0, 1, 2, 3).reshape(128, GROUPS * 128).astype(f8)

    b2bc = np.tile(b2, 4).reshape(128, 1).astype(np.float32)
    ident = np.eye(64, dtype=np.float32).astype(bf)

    # step-0 shock-source tiles (identical on every core)
    sh = shock.astype(np.int64)
    f0d_sh = np.abs(f0[sh][:, None] - f0[None, :])          # [4, 512]
    S0 = np.empty((128, 2 * N), np.float32)
    for pair in range(2):
        for half in range(2):
            r = sh[2 * pair + half]
            rows = slice(64 * half, 64 * half + 64)
            S0[rows, pair * N:(pair + 1) * N] = (
                B.T + np.outer(w_cg, cg[r]) + np.outer(w_f, f0d_sh[2 * pair + half]))
    S0 = S0.astype(bf)
    c_sh = A[sh] + b1[None, :] + 1.0 * w_p[None, :]          # [4, 64]
    bias0 = np.empty((128, 2), np.float32)
    for pair in range(2):
        bias0[0:64, pair] = c_sh[2 * pair]
        bias0[64:128, pair] = c_sh[2 * pair + 1]
    pc0 = cg[sh].astype(bf)                                  # [4, 512]
    LW30 = np.zeros((128, 4), np.float32)
    for q in range(4):
        LW30[32 * q:32 * (q + 1), q] = W3[:, 0]
    LW30 = LW30.astype(bf)

    in_maps = []
    for d in range(N_CORES):
        own = np.array([128 * (l % 4) + 16 * d + l // 4 for l in range(ROWS)])
        cg_d = cg[own]                   # [64, 512]
        A_d = A[own]                     # [64, 64]
        f0_d = f0[own]

        S_pack = np.empty((128, PAIRS * N), np.float32)
        BT = B.T                         # [D, N]
        f0dT = np.abs(f0_d[:, None] - f0[None, :])   # [64, 512]
        for i2 in range(PAIRS):
            lo, hi = i2, i2 + 32
            blk = slice(i2 * N, (i2 + 1) * N)
            S_pack[0:64, blk] = BT + np.outer(w_cg, cg_d[lo]) + np.outer(w_f, f0dT[lo])
            S_pack[64:128, blk] = BT + np.outer(w_cg, cg_d[hi]) + np.outer(w_f, f0dT[hi])
        S_bf = S_pack[:, 0:20 * N].astype(bf)
        S_f8 = S_pack[:, 20 * N:32 * N].astype(f8)

        Ab1s = np.empty((128, 32 * STEPS), np.float32)
        for s in range(STEPS):
            base = b1[None, :] + (np.float32(s) / np.float32(STEPS)) * w_s[None, :]
            blk = slice(32 * s, 32 * (s + 1))
            Ab1s[0:64, blk] = (A_d[0:32] + base).T
            Ab1s[64:128, blk] = (A_d[32:64] + base).T
        wp2 = np.zeros((2, 128), np.float32)
        wp2[0, 0:64] = w_p
        wp2[1, 64:128] = w_p

        pcol0 = p0[own].reshape(64, 1).astype(np.float32)
        arr0c = arr0[own].reshape(64, 1).astype(np.float32)
        p20 = p0[own].reshape(2, 32).astype(np.float32)

        in_maps.append({
            "S_in": S_bf, "S8_in": S_f8, "W2blk_in": W2blk, "W2dr_in": W2dr,
            "S0_in": S0, "bias0_in": bias0, "pc0_in": pc0, "LW30_in": LW30,
            "LW3_in": LW3, "Ab1s_in": Ab1s, "wp2_in": wp2,
            "cg_in": cg_d.astype(bf), "b2bc_in": b2bc,
            "b3bc_in": np.full((64, 1), b3, np.float32),
            "ident_in": ident,
            "pcol0_in": pcol0, "p20_in": p20, "arr0_in": arr0c,
        })
    return in_maps


def kernel(**inputs):
    from concourse.bass_utils import run_bass_kernel_spmd

    in_maps = _host_prep(inputs)
    if "nc" not in _CACHE:
        _CACHE["nc"] = _build_bass()
    nc = _CACHE["nc"]

    res = run_bass_kernel_spmd(nc, in_maps, core_ids=list(range(N_CORES)))
    p_full = np.empty(N, np.float32)
    arr_full = np.empty(N, np.float32)
    for d in range(N_CORES):
        own = np.array([128 * (l % 4) + 16 * d + l // 4 for l in range(ROWS)])
        p_full[own] = res.results[d]["p_out"]
        arr_full[own] = res.results[d]["arr_out"]
    arr_full = np.where(arr_full >= BIG / 2, np.inf, arr_full).astype(np.float32)
    return p_full, arr_full
